# revision 1
# baseline (speedup 1.0000x reference)
"""GeAT layer (graph attention w/ per-edge MLP scoring) on 8 Trainium2 cores.

Strategy (fully sparse — the dense [H,N,N] tensor is never materialized):
  - Directed edges (symmetric doubling of the input edge list) are deduped
    (last-write-wins, matching XLA scatter-set) and sharded by SOURCE row:
    core c owns rows [c*512, (c+1)*512), i.e. all softmax rows it outputs.
    Fully data-parallel: no cross-core communication at all.
  - Host prep is index/layout work plus parameter fusion: per-edge gathered
    embeddings emb[src], emb[dst] are shipped transposed as one
    [128, E_core] operand, edges sorted by (row-block, bond) with padding so
    every core runs the identical SPMD program. Qw/Kw/Qb/Kb are fused into
    the first MLP layer's weights (h0 = relu([emb_s emb_d] @ [[Qw@W0t],
    [Kw@W0b]] + b0')) so no separate Q/K projection stage is needed.
  - On device, per core: per-bond 2-layer MLP on raw edge embeddings (heads
    packed in pairs of 64 into the 128-wide PE array) -> leaky-relu ->
    per-(rowblock,head) max-subtracted exp -> segment softmax-aggregate
    expressed as PSUM-accumulated matmuls against an iota==srcrel 0/1 mask
    (the "scatter") with w-scaled V rows (the "gather") -> final projection.
  - The edge-embedding stream, MLP weights, scatter mask and aggregation
    operands are bf16 (exact for the 0/1 mask; the kernel is HBM-bound on
    the per-edge stream); scores and softmax weights/normalization stay
    f32/float32r. Emission is software-pipelined at half-row-block
    granularity so the PE/ACT MLP stream of one unit overlaps the
    DVE/GPSIMD softmax-aggregate tail of the previous one.
"""

import sys

sys.path.insert(0, "/opt/trn_rl_repo")

import numpy as np

N, D, H, B, HID = 4096, 64, 4, 4, 64
NEG = 0.2
C = 8            # cores
RPC = N // C     # rows per core
NRB = 4          # row blocks per core
RBS = 128        # rows per block

_cache = {}


def _host_prep(embeddings, src, dst, bond):
    emb = np.ascontiguousarray(np.asarray(embeddings, np.float32))
    src = np.asarray(src).astype(np.int64)
    dst = np.asarray(dst).astype(np.int64)
    bond = np.asarray(bond).astype(np.int64)

    s_all = np.concatenate([src, dst])
    d_all = np.concatenate([dst, src])
    b_all = np.concatenate([bond, bond])
    L = s_all.shape[0]

    # scatter-set duplicate resolution: last occurrence wins
    key = s_all * N + d_all
    order = np.argsort(key, kind="stable")
    ks = key[order]
    is_last = np.ones(L, bool)
    is_last[:-1] = ks[1:] != ks[:-1]
    alive = np.zeros(L, bool)
    alive[order[is_last]] = True

    core = s_all // RPC
    rb = (s_all % RPC) // RBS
    srel = (s_all % RBS).astype(np.float32)

    counts = np.zeros((C, NRB, B), np.int64)
    np.add.at(counts, (core[alive], rb[alive], b_all[alive]), 1)
    Lb = [int(-(-counts[:, :, b].max() // 128) * 128) for b in range(B)]
    offs = np.concatenate([[0], np.cumsum(Lb)]).astype(np.int64)
    R = int(offs[-1])
    ERUN = NRB * R
    NTILE = ERUN // 128

    xembT = np.zeros((C, 128, ERUN), np.float32)
    srcrel = np.full((C, 128, NTILE), -1.0, np.float32)
    bondslot = np.zeros((C, 128, NTILE), np.int64)
    for c in range(C):
        for r in range(NRB):
            for b in range(B):
                sel = np.where(alive & (core == c) & (rb == r) & (b_all == b))[0]
                lo = r * R + int(offs[b])
                allslots = lo + np.arange(Lb[b])
                bondslot[c, allslots % 128, allslots // 128] = b
                if len(sel) == 0:
                    continue
                slots = lo + np.arange(len(sel))
                xembT[c, 0:64, slots] = emb[s_all[sel]]
                xembT[c, 64:128, slots] = emb[d_all[sel]]
                srcrel[c, slots % 128, slots // 128] = srel[sel]
    return xembT, srcrel, bondslot, Lb, R


def _weights_prep(inp):
    f32 = np.float32
    Qw, Qb = np.asarray(inp["Qw"], f32), np.asarray(inp["Qb"], f32)
    Kw, Kb = np.asarray(inp["Kw"], f32), np.asarray(inp["Kb"], f32)
    Vw, Vb = np.asarray(inp["Vw"], f32), np.asarray(inp["Vb"], f32)
    W0, b0 = np.asarray(inp["W0"], f32), np.asarray(inp["b0"], f32)
    W1, b1 = np.asarray(inp["W1"], f32), np.asarray(inp["b1"], f32)
    W2, b2 = np.asarray(inp["W2"], f32), np.asarray(inp["b2"], f32)
    Pw, Pb = np.asarray(inp["Pw"], f32), np.asarray(inp["Pb"], f32)

    z = np.zeros((64, 64), f32)
    vwpad = np.concatenate([z, Vw], 0)                    # [128, 64]

    # fuse the Q/K projections into the first MLP layer (per bond, head)
    fw0 = np.zeros((B, H, 128, HID), f32)
    fb0 = np.zeros((B, H, HID), f32)
    for b in range(B):
        for h in range(H):
            fw0[b, h, 0:64] = Qw @ W0[b, h, 0:64]
            fw0[b, h, 64:128] = Kw @ W0[b, h, 64:128]
            fb0[b, h] = Qb @ W0[b, h, 0:64] + Kb @ W0[b, h, 64:128] + b0[b, h]

    w0all = np.zeros((128, B * 2 * 128), f32)
    w1all = np.zeros((128, B * 2 * 128), f32)
    w2all = np.zeros((128, B * 2 * 2), f32)
    b0all = np.zeros((128, B * 2), f32)
    b1all = np.zeros((128, B * 2), f32)
    b2all = np.zeros((2, B * 2), f32)
    for b in range(B):
        for pr in range(2):
            i = b * 2 + pr
            ha, hb = 2 * pr, 2 * pr + 1
            w0all[:, i * 128: i * 128 + 64] = fw0[b, ha]
            w0all[:, i * 128 + 64: (i + 1) * 128] = fw0[b, hb]
            w1all[0:64, i * 128: i * 128 + 64] = W1[b, ha]
            w1all[64:128, i * 128 + 64: (i + 1) * 128] = W1[b, hb]
            w2all[0:64, i * 2] = W2[b, ha]
            w2all[64:128, i * 2 + 1] = W2[b, hb]
            b0all[0:64, i] = fb0[b, ha]
            b0all[64:128, i] = fb0[b, hb]
            b1all[0:64, i] = b1[b, ha]
            b1all[64:128, i] = b1[b, hb]
            b2all[0, i] = b2[b, ha]
            b2all[1, i] = b2[b, hb]

    pw4 = np.zeros((64, H * 64), f32)                     # lhsT per head
    for h in range(H):
        pw4[:, h * 64:(h + 1) * 64] = Pw[h * 64:(h + 1) * 64]
    biascol = (Pb + np.tile(Vb, H) @ Pw)[:, None]         # [64, 1]

    iota = np.tile(np.arange(128, dtype=f32), (128, 1))   # [128, 128]
    id128 = np.eye(128, dtype=f32)

    return dict(vwpad=vwpad, w0all=w0all, w1all=w1all, w2all=w2all,
                b0all=b0all, b1all=b1all, b2all=b2all,
                pw4=pw4, biascol=biascol,
                iota=iota, id128=id128)


def _chunks(n, step=512):
    out = []
    s = 0
    while s < n:
        out.append((s, min(step, n - s)))
        s += step
    return out


def _build_program(Lb, R, loop=0):
    import concourse.bacc as bacc
    import concourse.tile as tile
    from concourse import mybir
    from contextlib import ExitStack

    f32 = mybir.dt.float32
    fr = mybir.dt.float32r
    bf = mybir.dt.bfloat16
    AF = mybir.ActivationFunctionType
    ALU = mybir.AluOpType

    ERUN = NRB * R
    NTILE = ERUN // 128
    TPB = R // 128
    offs = np.concatenate([[0], np.cumsum(Lb)]).astype(np.int64)
    TA = int(offs[2]) // 128           # tiles in bond group A = {0, 1}
    GRP = [(0, [0, 1], 0, TA), (1, [2, 3], TA, TPB - TA)]
    NG = len(GRP)

    # packed constant layouts (column offsets)
    CPK = {}
    o = 0
    for nm, w in [("srcrel", NTILE), ("iota", 128), ("biascol", 1)]:
        CPK[nm] = (o, w); o += w
    CPKW = o
    BPK = {}
    o = 0
    for nm, w in [("b0all", B * 2), ("b1all", B * 2)]:
        BPK[nm] = (o, w); o += w
    BPKW = o
    WBF = {}
    o = 0
    for nm, w in [("vwpad", 64), ("w2all", B * 2 * 2)]:
        WBF[nm] = (o, w); o += w
    WBFW = o
    WPK = {}
    o = 0
    for nm, w in [("pw4", H * 64), ("id128", 128), ("w2fr", B * 2 * 2)]:
        WPK[nm] = (o, w); o += w
    WPKW = o

    nc = bacc.Bacc("TRN2", target_bir_lowering=False, debug=False, num_devices=C)

    dspec = [("xembT", (128, ERUN), bf),
             ("w0b0", (128, 256), bf), ("w0r", (128, 768), bf),
             ("w1b0", (128, 256), bf), ("w1r", (128, 768), bf),
             ("bpk", (128, BPKW), f32), ("wbf", (128, WBFW), bf),
             ("wpkt", (128, WPKW), fr),
             ("cpk", (128, CPKW), f32), ("b2e", (128, NTILE * H), f32)]
    dram = {}
    for nm, shp, dt in dspec:
        dram[nm] = nc.dram_tensor(nm, list(shp), dt, kind="ExternalInput").ap()
    outT = nc.dram_tensor("outT", [64, RPC], f32, kind="ExternalOutput").ap()

    with ExitStack() as ctx:
        tc = ctx.enter_context(tile.TileContext(nc))
        constp = ctx.enter_context(tc.tile_pool(name="const", bufs=1))
        xep = ctx.enter_context(tc.tile_pool(name="xe", bufs=3))
        hidp = ctx.enter_context(tc.tile_pool(name="hid", bufs=4))
        vgp = ctx.enter_context(tc.tile_pool(name="vg", bufs=2))
        wtep = ctx.enter_context(tc.tile_pool(name="wte", bufs=2))
        mrbp = ctx.enter_context(tc.tile_pool(name="mrb", bufs=2))
        rhsp = ctx.enter_context(tc.tile_pool(name="rhs", bufs=2))
        aggsp = ctx.enter_context(tc.tile_pool(name="aggs", bufs=2))
        ohp = ctx.enter_context(tc.tile_pool(name="oh", bufs=2))
        finp = ctx.enter_context(tc.tile_pool(name="fin", bufs=1))
        psh0p = ctx.enter_context(tc.tile_pool(name="psh0", bufs=2, space="PSUM"))
        psh1p = ctx.enter_context(tc.tile_pool(name="psh1", bufs=2, space="PSUM"))
        psmixp = ctx.enter_context(tc.tile_pool(name="psmix", bufs=2, space="PSUM"))
        psaggp = ctx.enter_context(tc.tile_pool(name="psagg", bufs=2, space="PSUM"))

        def _emit_all():
            # DMA order tuned so bond-0 compute of row-block 0 starts early
            bpk = constp.tile([128, BPKW], f32, tag="bpk", name="bpk")
            nc.sync.dma_start(out=bpk[:], in_=dram["bpk"][:])
            w0b0 = constp.tile([128, 256], bf, tag="w0b0", name="w0b0")
            nc.sync.dma_start(out=w0b0[:], in_=dram["w0b0"][:])
            xe0b = []
            for b in range(B):
                t = xep.tile([128, Lb[b]], bf, tag=f"xe0b{b}", name=f"xe0b{b}",
                             bufs=1)
                xe0b.append(t)
            nc.sync.dma_start(out=xe0b[0][:], in_=dram["xembT"][:, 0:Lb[0]])
            w1b0 = constp.tile([128, 256], bf, tag="w1b0", name="w1b0")
            nc.sync.dma_start(out=w1b0[:], in_=dram["w1b0"][:])
            w0r = constp.tile([128, 768], bf, tag="w0r", name="w0r")
            nc.sync.dma_start(out=w0r[:], in_=dram["w0r"][:])
            nc.sync.dma_start(out=xe0b[1][:],
                              in_=dram["xembT"][:, int(offs[1]):int(offs[2])])
            w1r = constp.tile([128, 768], bf, tag="w1r", name="w1r")
            nc.sync.dma_start(out=w1r[:], in_=dram["w1r"][:])
            wbf = constp.tile([128, WBFW], bf, tag="wbf", name="wbf")
            nc.sync.dma_start(out=wbf[:], in_=dram["wbf"][:])
            wpkt = constp.tile([128, WPKW], fr, tag="wpkt", name="wpkt")
            nc.sync.dma_start(out=wpkt[:], in_=dram["wpkt"][:])
            nc.sync.dma_start(out=xe0b[2][:],
                              in_=dram["xembT"][:, int(offs[2]):int(offs[3])])
            nc.sync.dma_start(out=xe0b[3][:],
                              in_=dram["xembT"][:, int(offs[3]):int(offs[4])])
            cpk = constp.tile([128, CPKW], f32, tag="cpk", name="cpk")
            nc.sync.dma_start(out=cpk[:], in_=dram["cpk"][:])
            b2esb = constp.tile([128, NTILE, H], f32, tag="b2e", name="b2e")
            nc.sync.dma_start(
                out=b2esb[:],
                in_=dram["b2e"][:].rearrange("p (t h) -> p t h", h=H))
            xes = [None]
            for rb in range(1, NRB):
                t = xep.tile([128, R], bf, tag="xe", name="xe")
                nc.sync.dma_start(
                    out=t[:], in_=dram["xembT"][:, rb * R:(rb + 1) * R])
                xes.append(t)

            def cp(nm):
                o, w = CPK[nm]
                return cpk[:, o:o + w]

            def bp(nm):
                o, w = BPK[nm]
                return bpk[:, o:o + w]

            def wp(nm):
                o, w = WPK[nm]
                return wpkt[:, o:o + w]

            def wb(nm):
                o, w = WBF[nm]
                return wbf[:, o:o + w]

            def w0_ap(b):
                return w0b0 if b == 0 else w0r[:, (b - 1) * 256:b * 256]

            def w1_ap(b):
                return w1b0 if b == 0 else w1r[:, (b - 1) * 256:b * 256]

            def xe_ap(rb, lo, ln):
                if rb == 0:
                    b = int(np.searchsorted(offs, lo, side="right") - 1)
                    return xe0b[b][:, lo - int(offs[b]): lo - int(offs[b]) + ln]
                return xes[rb][:, lo:lo + ln]

            psAs = {}

            def emit_head(rb, g, bonds, t0, tn):
                """MLP for this bond group; L3 -> edge-major psE; then Vg."""
                psE = psmixp.tile([128, tn * 4], f32, tag="mix",
                                  name=f"psE{g}")
                for b in bonds:
                    if Lb[b] == 0:
                        continue
                    for (cs, cl) in _chunks(Lb[b]):
                        lo = int(offs[b]) + cs
                        for pr in range(2):
                            i = b * 2 + pr
                            p0 = psh0p.tile([128, 512], f32, tag="h0", name="p0")
                            nc.tensor.matmul(
                                p0[:, :cl],
                                lhsT=w0_ap(b)[:, pr * 128:(pr + 1) * 128],
                                rhs=xe_ap(rb, lo, cl),
                                start=True, stop=True)
                            h0 = hidp.tile([128, 512], bf, tag="h0s", name="h0")
                            nc.scalar.activation(h0[:, :cl], p0[:, :cl], AF.Relu,
                                                 bias=bp("b0all")[:, i:i + 1])
                            p1 = psh1p.tile([128, 512], f32, tag="h1", name="p1")
                            nc.tensor.matmul(
                                p1[:, :cl],
                                lhsT=w1_ap(b)[:, pr * 128:(pr + 1) * 128],
                                rhs=h0[:, :cl],
                                start=True, stop=True)
                            dve_relu = (pr == 1 and b >= 2)
                            if dve_relu:
                                h1 = hidp.tile([128, 512], fr, tag="h1f",
                                               name="h1f")
                                nc.vector.tensor_scalar(
                                    out=h1[:, :cl], in0=p1[:, :cl],
                                    scalar1=bp("b1all")[:, i:i + 1],
                                    scalar2=0.0, op0=ALU.add, op1=ALU.max)
                                w2 = wp("w2fr")
                            else:
                                h1 = hidp.tile([128, 512], bf, tag="h1s",
                                               name="h1")
                                nc.scalar.activation(h1[:, :cl], p1[:, :cl],
                                                     AF.Relu,
                                                     bias=bp("b1all")[:, i:i + 1])
                                w2 = wb("w2all")
                            for j in range(cl // 128):
                                sl = (lo // 128) + j - t0
                                nc.tensor.matmul(
                                    psE[:, sl * 4 + pr * 2: sl * 4 + pr * 2 + 2],
                                    lhsT=h1[:, j * 128:(j + 1) * 128],
                                    rhs=w2[:, i * 2:(i + 1) * 2],
                                    start=True, stop=True)

                # V rows for this group's tiles
                vg = vgp.tile([128, tn, 64], fr, tag="vg", name="vg")
                for q0 in range(0, tn, 8):
                    qn = min(8, tn - q0)
                    pv = psaggp.tile([128, 512], f32, tag="agg", name="pv")
                    for q in range(q0, q0 + qn):
                        k = (q - q0) * 64
                        nc.tensor.matmul(pv[:, k:k + 64],
                                         lhsT=xe_ap(rb, (t0 + q) * 128, 128),
                                         rhs=wb("vwpad"),
                                         start=True, stop=True)
                    nc.vector.tensor_copy(
                        vg[:, q0:q0 + qn, :],
                        pv[:, :qn * 64].rearrange("p (t f) -> p t f", f=64))
                return vg, psE

            def emit_tail(rb, g, bonds, t0, tn, vg, psE):
                wte = wtep.tile([128, tn, 4], f32, tag="wte", name="wte")
                nc.vector.tensor_copy(wte[:],
                                      psE[:].rearrange("p (t f) -> p t f", f=4))
                nc.vector.tensor_tensor(
                    out=wte[:], in0=wte[:],
                    in1=b2esb[:, rb * TPB + t0: rb * TPB + t0 + tn, :],
                    op=ALU.add)
                wl = wtep.tile([128, tn, 4], f32, tag="wl", name="wl", bufs=1)
                nc.vector.tensor_scalar_mul(wl[:], wte[:], NEG)
                nc.vector.tensor_tensor(out=wte[:], in0=wte[:], in1=wl[:],
                                        op=ALU.max)
                nc.scalar.activation(wte[:], wte[:], AF.Exp)

                mrb = mrbp.tile([128, tn, 128], bf, tag="mrb", name="mrb")
                nc.vector.tensor_tensor(
                    out=mrb[:],
                    in0=cp("iota").unsqueeze(1).to_broadcast([128, tn, 128]),
                    in1=cp("srcrel")[:, rb * TPB + t0: rb * TPB + t0 + tn]
                        .unsqueeze(2).to_broadcast([128, tn, 128]),
                    op=ALU.is_equal)

                rhs = rhsp.tile([128, tn, 4 * 65], bf, tag="rhs", name="rhs")
                for h in range(H):
                    eng = nc.vector if h < 2 else nc.gpsimd
                    eng.tensor_tensor(
                        out=rhs[:, :, h * 65: h * 65 + 64],
                        in0=vg[:],
                        in1=wte[:, :, h:h + 1].to_broadcast([128, tn, 64]),
                        op=ALU.mult)
                    nc.vector.tensor_copy(rhs[:, :, h * 65 + 64: h * 65 + 65],
                                          wte[:, :, h:h + 1])

                if g == 0:
                    psAs[rb] = psaggp.tile([128, 4 * 65], f32, tag="agg",
                                           name="psA")
                psA = psAs[rb]
                for q in range(tn):
                    nc.tensor.matmul(psA[:],
                                     lhsT=mrb[:, q, :],
                                     rhs=rhs[:, q, :],
                                     start=(g == 0 and q == 0),
                                     stop=(g == NG - 1 and q == tn - 1))
                if g != NG - 1:
                    return

                aggsb = aggsp.tile([128, 4 * 65], f32, tag="aggsb", name="aggsb")
                nc.vector.tensor_copy(aggsb[:], psA[:])
                rz = ohp.tile([128, H], f32, tag="rz", name="rz", bufs=1)
                nc.vector.reciprocal(
                    rz[:], aggsb[:].rearrange("p (h z) -> p h z", z=65)[:, :, 64])
                oh = ohp.tile([128, H, 64], fr, tag="oh", name="oh")
                for h in range(H):
                    nc.vector.tensor_tensor(
                        out=oh[:, h, :],
                        in0=aggsb[:, h * 65: h * 65 + 64],
                        in1=rz[:, h:h + 1].to_broadcast([128, 64]),
                        op=ALU.mult)
                po = psaggp.tile([64, 512], fr, tag="agg", name="po")
                for h in range(H):
                    nc.tensor.transpose(out=po[:, h * 128:(h + 1) * 128],
                                        in_=oh[:, h, :],
                                        identity=wp("id128"))
                otrb = ohp.tile([64, H, 128], fr, tag="otrb", name="otrb")
                for h in range(H):
                    nc.vector.tensor_copy(otrb[:, h, :],
                                          po[:, h * 128:(h + 1) * 128])
                # project this row-block and ship it out immediately
                psP = psmixp.tile([64, 128], f32, tag="mix", name="psP")
                for h in range(H):
                    nc.tensor.matmul(psP[:],
                                     lhsT=wp("pw4")[0:64, h * 64:(h + 1) * 64],
                                     rhs=otrb[:, h, :],
                                     start=(h == 0), stop=(h == H - 1))
                outsb = finp.tile([64, 128], f32, tag="outsb", name="outsb",
                                  bufs=2)
                nc.vector.tensor_tensor(
                    out=outsb[:], in0=psP[:],
                    in1=cp("biascol")[0:64, :].to_broadcast([64, 128]),
                    op=ALU.add)
                nc.sync.dma_start(out=outT[:, rb * 128:(rb + 1) * 128],
                                  in_=outsb[:])

            # software pipeline at bond-group granularity: tail(u) follows
            # head(u+1), so every tail overlaps the next group's MLP stream
            units = [(rb, *grp) for rb in range(NRB) for grp in GRP]
            pend = None
            for u in units:
                rb, g, bonds, t0, tn = u
                hnd = emit_head(rb, g, bonds, t0, tn)
                if pend is not None:
                    (prb, pg, pbonds, pt0, ptn), ph = pend
                    emit_tail(prb, pg, pbonds, pt0, ptn, *ph)
                pend = (u, hnd)
            (prb, pg, pbonds, pt0, ptn), ph = pend
            emit_tail(prb, pg, pbonds, pt0, ptn, *ph)


        if loop:
            with tc.For_i(0, loop, 1):
                _emit_all()
        else:
            _emit_all()

    nc.compile()
    return nc


def _prepare(inputs):
    import ml_dtypes
    bf16 = ml_dtypes.bfloat16
    xembT, srcrel, bondslot, Lb, R = _host_prep(
        inputs["embeddings"], inputs["src"], inputs["dst"], inputs["bond"])
    wts = _weights_prep(inputs)
    b2 = np.asarray(inputs["b2"], np.float32)          # [B, H]
    b2e = b2[bondslot]                                  # [C, 128, NTILE, H]
    NTILE = (NRB * R) // 128
    f32 = np.float32

    cpk = np.zeros((128, NTILE + 128 + 1), f32)
    o = 0
    o_srcrel = o; o += NTILE
    cpk[:, o:o + 128] = wts["iota"]; o += 128
    cpk[0:64, o:o + 1] = wts["biascol"]; o += 1

    bpk = np.zeros((128, B * 2 + B * 2), f32)
    o = 0
    bpk[:, o:o + B * 2] = wts["b0all"]; o += B * 2
    bpk[:, o:o + B * 2] = wts["b1all"]; o += B * 2

    wbf = np.zeros((128, 64 + B * 2 * 2), bf16)
    o = 0
    wbf[:, o:o + 64] = wts["vwpad"].astype(bf16); o += 64
    wbf[:, o:o + B * 2 * 2] = wts["w2all"].astype(bf16); o += B * 2 * 2

    wpkt = np.zeros((128, H * 64 + 128 + B * 2 * 2), f32)
    o = 0
    wpkt[0:64, o:o + H * 64] = wts["pw4"]; o += H * 64
    wpkt[:, o:o + 128] = wts["id128"]; o += 128
    wpkt[:, o:o + B * 2 * 2] = wts["w2all"]; o += B * 2 * 2

    w0b0 = np.ascontiguousarray(wts["w0all"][:, 0:256]).astype(bf16)
    w0r = np.ascontiguousarray(wts["w0all"][:, 256:1024]).astype(bf16)
    w1b0 = np.ascontiguousarray(wts["w1all"][:, 0:256]).astype(bf16)
    w1r = np.ascontiguousarray(wts["w1all"][:, 256:1024]).astype(bf16)

    key = (tuple(Lb), R)
    if key not in _cache:
        _cache.clear()
        _cache[key] = _build_program(Lb, R)
    nc = _cache[key]
    in_maps = []
    for c in range(C):
        cpkc = cpk.copy()
        cpkc[:, o_srcrel:o_srcrel + NTILE] = srcrel[c]
        m = {"xembT": xembT[c].astype(bf16), "b2e": b2e[c].reshape(128, -1),
             "w0b0": w0b0, "w0r": w0r, "w1b0": w1b0, "w1r": w1r,
             "bpk": bpk, "wbf": wbf, "wpkt": wpkt, "cpk": cpkc}
        in_maps.append(m)
    return nc, in_maps


def kernel(**inputs):
    from concourse.bass_utils import run_bass_kernel_spmd

    nc, in_maps = _prepare(inputs)
    res = run_bass_kernel_spmd(nc, in_maps, list(range(C)))
    out = np.empty((N, D), np.float32)
    for c in range(C):
        out[c * RPC:(c + 1) * RPC] = res.results[c]["outT"].T
    return out


def benchmark(inputs, iters=10, warmup=2):
    """Time repeated executions of the compiled SPMD program with
    device-resident inputs (excludes compile and host<->device transfer)."""
    import time
    import jax
    from jax.experimental.shard_map import shard_map
    from jax.sharding import Mesh, PartitionSpec, NamedSharding
    from concourse import bass2jax as b2j
    from concourse import mybir

    nc, in_maps = _prepare(inputs)
    b2j.install_neuronx_cc_hook()
    partition_name = nc.partition_id_tensor.name if nc.partition_id_tensor else None
    in_names, out_names, out_avals, zero_outs = [], [], [], []
    for alloc in nc.m.functions[0].allocations:
        if not isinstance(alloc, mybir.MemoryLocationSet):
            continue
        name = alloc.memorylocations[0].name
        if alloc.kind == "ExternalInput":
            if name != partition_name:
                in_names.append(name)
        elif alloc.kind == "ExternalOutput":
            out_names.append(name)
            shape = tuple(alloc.tensor_shape)
            dtype = mybir.dt.np(alloc.dtype)
            out_avals.append(jax.core.ShapedArray(shape, dtype))
            zero_outs.append(np.zeros(shape, dtype))
    n_params = len(in_names)
    all_in = in_names + out_names + ([partition_name] if partition_name else [])
    donate = tuple(range(n_params, n_params + len(out_names)))

    def _body(*args):
        operands = list(args)
        if partition_name is not None:
            operands.append(b2j.partition_id_tensor())
        outs = b2j._bass_exec_p.bind(
            *operands, out_avals=tuple(out_avals), in_names=tuple(all_in),
            out_names=tuple(out_names), lowering_input_output_aliases=(),
            sim_require_finite=True, sim_require_nnan=True, nc=nc)
        return tuple(outs)

    devices = jax.devices()[:C]
    mesh = Mesh(np.asarray(devices), ("core",))
    in_specs = (PartitionSpec("core"),) * (n_params + len(out_names))
    out_specs = (PartitionSpec("core"),) * len(out_names)
    sharded = jax.jit(shard_map(_body, mesh=mesh, in_specs=in_specs,
                                out_specs=out_specs, check_rep=False),
                      donate_argnums=donate, keep_unused=True)
    sh = NamedSharding(mesh, PartitionSpec("core"))
    concat_in = [
        jax.device_put(
            np.concatenate([np.asarray(in_maps[c][n]) for c in range(C)], axis=0), sh)
        for n in in_names]

    times = []
    for it in range(warmup + iters):
        zs = [jax.device_put(np.zeros((C * z.shape[0], *z.shape[1:]), z.dtype), sh)
              for z in zero_outs]
        t0 = time.perf_counter()
        out = sharded(*concat_in, *zs)
        jax.block_until_ready(out)
        dt = time.perf_counter() - t0
        if it >= warmup:
            times.append(dt)
    print("bench times (ms):", [f"{t*1e3:.3f}" for t in times])
    return min(times) * 1e9


def benchmark_hw(inputs, k=512, iters=6, warmup=2, k_small=None):
    """Real-HW timing: run the whole per-core program k times inside one
    NEFF (tc.For_i) and wall-time it through the tunnel. If k_small is
    given, also times a k_small-loop NEFF and returns the difference
    quotient, which cancels the (~80ms) tunnel dispatch floor exactly."""
    if k_small:
        t_big = benchmark_hw(inputs, k=k, iters=iters, warmup=warmup)
        t_sml = benchmark_hw(inputs, k=k_small, iters=iters, warmup=warmup)
        return (t_big * k - t_sml * k_small) / (k - k_small)
    import time
    import jax
    from jax.experimental.shard_map import shard_map
    from jax.sharding import Mesh, PartitionSpec, NamedSharding
    from concourse import bass2jax as b2j
    from concourse import mybir

    xembT, srcrel, bondslot, Lb, R = _host_prep(
        inputs["embeddings"], inputs["src"], inputs["dst"], inputs["bond"])
    nc0, in_maps = _prepare(inputs)
    nc = _build_program(Lb, R, loop=k)

    b2j.install_neuronx_cc_hook()
    partition_name = nc.partition_id_tensor.name if nc.partition_id_tensor else None
    in_names, out_names, out_avals, zero_outs = [], [], [], []
    for alloc in nc.m.functions[0].allocations:
        if not isinstance(alloc, mybir.MemoryLocationSet):
            continue
        name = alloc.memorylocations[0].name
        if alloc.kind == "ExternalInput":
            if name != partition_name:
                in_names.append(name)
        elif alloc.kind == "ExternalOutput":
            out_names.append(name)
            shape = tuple(alloc.tensor_shape)
            dtype = mybir.dt.np(alloc.dtype)
            out_avals.append(jax.core.ShapedArray(shape, dtype))
            zero_outs.append(np.zeros(shape, dtype))
    n_params = len(in_names)
    all_in = in_names + out_names + ([partition_name] if partition_name else [])
    donate = tuple(range(n_params, n_params + len(out_names)))

    def _body(*args):
        operands = list(args)
        if partition_name is not None:
            operands.append(b2j.partition_id_tensor())
        outs = b2j._bass_exec_p.bind(
            *operands, out_avals=tuple(out_avals), in_names=tuple(all_in),
            out_names=tuple(out_names), lowering_input_output_aliases=(),
            sim_require_finite=True, sim_require_nnan=True, nc=nc)
        return tuple(outs)

    devices = jax.devices()[:C]
    mesh = Mesh(np.asarray(devices), ("core",))
    in_specs = (PartitionSpec("core"),) * (n_params + len(out_names))
    out_specs = (PartitionSpec("core"),) * len(out_names)
    sharded = jax.jit(shard_map(_body, mesh=mesh, in_specs=in_specs,
                                out_specs=out_specs, check_rep=False),
                      donate_argnums=donate, keep_unused=True)
    sh = NamedSharding(mesh, PartitionSpec("core"))
    concat_in = [
        jax.device_put(
            np.concatenate([np.asarray(in_maps[c][n]) for c in range(C)], axis=0),
            sh)
        for n in in_names]
    times = []
    for it in range(warmup + iters):
        zs = [jax.device_put(np.zeros((C * z.shape[0], *z.shape[1:]), z.dtype), sh)
              for z in zero_outs]
        t0 = time.perf_counter()
        out = sharded(*concat_in, *zs)
        jax.block_until_ready(out)
        dt = time.perf_counter() - t0
        if it >= warmup:
            times.append(dt)
    print("looped bench times (ms):", [f"{t*1e3:.2f}" for t in times])
    best = min(times)
    return best * 1e9 / k



# revision 62
# speedup vs baseline: 1.1285x; 1.1285x over previous
"""GeAT layer (graph attention w/ per-edge MLP scoring) on 8 Trainium2 cores.

Strategy (fully sparse — the dense [H,N,N] tensor is never materialized):
  - Directed edges (symmetric doubling) are deduped (last-write-wins, matching
    XLA scatter-set) and rows are PERMUTED into 32 balanced buckets of 128
    (greedy load balancing on per-row per-bond edge counts), so every
    (core, rowblock, bond) slot range has the same near-minimal capacity.
    Fully data-parallel: no cross-core communication.
  - Host prep ships, per core: the edge stream feat-major ([128, E] = Qs/Kd
    halves), the dst-embedding stream edge-major (for the aggregation rhs),
    the 0/1 scatter mask (slot -> row-in-block), and the per-slot b2 bias.
    Qw/Kw/Qb/Kb are fused into the first MLP layer; Vw/Vb are fused into the
    output projection (aggregation runs on raw dst embeddings).
  - On device, per (rowblock, bondgroup) unit: per-bond 2-layer MLP (heads
    packed in pairs of 64) -> scores -> leaky/exp -> weight-scaled dst-emb
    rhs -> PSUM-accumulated mask matmuls (segment softmax-aggregate) ->
    normalize -> transpose -> fused projection. Elementwise work is spread
    across Act/DVE/Pool so the tensor engine is the critical resource.
"""

import sys

sys.path.insert(0, "/opt/trn_rl_repo")

import numpy as np

N, D, H, B, HID = 4096, 64, 4, 4, 64
NEG = 0.2
C = 8            # cores
RPC = N // C     # rows per core
NRB = 4          # row blocks per core
RBS = 128        # rows per block
NBUCK = C * NRB  # 32 row buckets

_cache = {}


def _balance_rows(cnt):
    """Assign 4096 rows to 32 buckets of 128, minimizing per-bond max."""
    nb = NBUCK
    target = cnt.sum(0) / nb
    order = np.argsort(-cnt.sum(1), kind="stable")
    sums = np.zeros((nb, B))
    sizes = np.zeros(nb, np.int64)
    assign = np.empty(N, np.int64)
    for r in order:
        cand = np.where(sizes < RBS)[0]
        news = sums[cand] + cnt[r]
        cost = (news / target).max(1) + 0.0005 * sizes[cand]
        k = cand[int(np.argmin(cost))]
        assign[r] = k
        sums[k] += cnt[r]
        sizes[k] += 1
    maxes = sums.max(0).astype(np.int64)
    # per-bond caps: round achieved maxes up to 32, pad the total to a
    # multiple of 128 (tile size), and keep every bond-start offset at
    # 0/32/64 mod 128 (PE psum writes can't start at partition 96)
    base = [int(-(-m // 32) * 32) for m in maxes]
    best = None
    import itertools
    for pads in itertools.product((0, 32, 64, 96, 128), repeat=B):
        Lb = [base[b] + pads[b] for b in range(B)]
        if sum(Lb) % RBS != 0:
            continue
        if any(sum(Lb[:b]) % RBS == 96 for b in range(1, B)):
            continue
        if best is None or sum(Lb) < sum(best):
            best = Lb
    return assign, best


def _host_prep(embeddings, src, dst, bond):
    emb = np.ascontiguousarray(np.asarray(embeddings, np.float32))
    src = np.asarray(src).astype(np.int64)
    dst = np.asarray(dst).astype(np.int64)
    bond = np.asarray(bond).astype(np.int64)

    s_all = np.concatenate([src, dst])
    d_all = np.concatenate([dst, src])
    b_all = np.concatenate([bond, bond])
    L = s_all.shape[0]

    # scatter-set duplicate resolution: last occurrence wins
    key = s_all * N + d_all
    order = np.argsort(key, kind="stable")
    ks = key[order]
    is_last = np.ones(L, bool)
    is_last[:-1] = ks[1:] != ks[:-1]
    alive = np.zeros(L, bool)
    alive[order[is_last]] = True

    cnt = np.zeros((N, B), np.int64)
    np.add.at(cnt, (s_all[alive], b_all[alive]), 1)
    assign, Lb = _balance_rows(cnt)

    R = int(sum(Lb))
    TPB = R // RBS
    NTILE = NRB * TPB
    ERUN = NRB * R
    offs = np.concatenate([[0], np.cumsum(Lb)]).astype(np.int64)

    # rows of bucket k in stable order; srel = index within bucket
    rowmap = np.empty((NBUCK, RBS), np.int64)
    srel_of = np.empty(N, np.int64)
    for k in range(NBUCK):
        rows = np.where(assign == k)[0]
        rowmap[k] = rows
        srel_of[rows] = np.arange(RBS)

    core_of = assign // NRB
    rb_of = assign % NRB

    ecore = core_of[s_all]
    erb = rb_of[s_all]
    esrel = srel_of[s_all]

    xembT = np.zeros((C, 128, ERUN), np.float32)
    maskh = np.zeros((C, 128, NRB, TPB * RBS), np.float32)
    vge = np.zeros((C, 128, NRB, 64, TPB), np.float32)
    b2slot = np.zeros((C, 128, NRB, TPB, H), np.int64)  # bond id per slot
    for c in range(C):
        for r in range(NRB):
            for b in range(B):
                sel = np.where(alive & (ecore == c) & (erb == r)
                               & (b_all == b))[0]
                lo = int(offs[b])
                allslots = lo + np.arange(Lb[b])
                b2slot[c, allslots % 128, r, allslots // 128, :] = b
                if len(sel) == 0:
                    continue
                slots = lo + np.arange(len(sel))
                p, t = slots % 128, slots // 128
                gs = r * R + slots
                xembT[c, 0:64, gs] = emb[s_all[sel]]
                xembT[c, 64:128, gs] = emb[d_all[sel]]
                maskh[c, p, r, t * RBS + esrel[sel]] = 1.0
                vge[c, p, r, :, t] = emb[d_all[sel]]
    return dict(xembT=xembT, maskh=maskh, vge=vge, b2slot=b2slot,
                rowmap=rowmap, Lb=tuple(int(x) for x in Lb), R=R,
                TPB=TPB, NTILE=NTILE, ERUN=ERUN)


def _weights_prep(inp):
    f32 = np.float32
    Qw, Qb = np.asarray(inp["Qw"], f32), np.asarray(inp["Qb"], f32)
    Kw, Kb = np.asarray(inp["Kw"], f32), np.asarray(inp["Kb"], f32)
    Vw, Vb = np.asarray(inp["Vw"], f32), np.asarray(inp["Vb"], f32)
    W0, b0 = np.asarray(inp["W0"], f32), np.asarray(inp["b0"], f32)
    W1, b1 = np.asarray(inp["W1"], f32), np.asarray(inp["b1"], f32)
    W2, b2 = np.asarray(inp["W2"], f32), np.asarray(inp["b2"], f32)
    Pw, Pb = np.asarray(inp["Pw"], f32), np.asarray(inp["Pb"], f32)

    # fuse the Q/K projections into the first MLP layer (per bond, head)
    fw0 = np.zeros((B, H, 128, HID), f32)
    fb0 = np.zeros((B, H, HID), f32)
    for b in range(B):
        for h in range(H):
            fw0[b, h, 0:64] = Qw @ W0[b, h, 0:64]
            fw0[b, h, 64:128] = Kw @ W0[b, h, 64:128]
            fb0[b, h] = Qb @ W0[b, h, 0:64] + Kb @ W0[b, h, 64:128] + b0[b, h]

    w0all = np.zeros((128, B * 2 * 128), f32)
    w1all = np.zeros((128, B * 2 * 128), f32)
    w2all = np.zeros((128, B * 2 * 2), f32)
    b0all = np.zeros((128, B * 2), f32)
    b1all = np.zeros((128, B * 2), f32)
    for b in range(B):
        for pr in range(2):
            i = b * 2 + pr
            ha, hb = 2 * pr, 2 * pr + 1
            w0all[:, i * 128: i * 128 + 64] = fw0[b, ha]
            w0all[:, i * 128 + 64: (i + 1) * 128] = fw0[b, hb]
            w1all[0:64, i * 128: i * 128 + 64] = W1[b, ha]
            w1all[64:128, i * 128 + 64: (i + 1) * 128] = W1[b, hb]
            w2all[0:64, i * 2] = W2[b, ha]
            w2all[64:128, i * 2 + 1] = W2[b, hb]
            b0all[0:64, i] = fb0[b, ha]
            b0all[64:128, i] = fb0[b, hb]
            b1all[0:64, i] = b1[b, ha]
            b1all[64:128, i] = b1[b, hb]

    # fold Vw into the projection: out = concat_h(aggemb_h) @ (Vw @ Pw_h) + bias
    pw4 = np.zeros((64, H * 64), f32)
    for h in range(H):
        pw4[:, h * 64:(h + 1) * 64] = Vw @ Pw[h * 64:(h + 1) * 64]
    biascol = (Pb + np.tile(Vb, H) @ Pw)[:, None]         # [64, 1]
    id128 = np.eye(128, dtype=f32)

    return dict(w0all=w0all, w1all=w1all, w2all=w2all,
                b0all=b0all, b1all=b1all, b2=b2,
                pw4=pw4, biascol=biascol, id128=id128)


def _chunks(n, step=512):
    out = []
    s = 0
    while s < n:
        out.append((s, min(step, n - s)))
        s += step
    return out


# packed constant column layouts
def _wbf_layout(TPB):
    WBF = {}
    o = 0
    for nm, w in [("w2all", B * 2 * 2), ("pw4", H * 64), ("id128", 128)]:
        WBF[nm] = (o, w)
        o += w
    return WBF, o


def _build_program(Lb, R, loop=0):
    import concourse.bacc as bacc
    import concourse.tile as tile
    from concourse import mybir
    from contextlib import ExitStack

    f32 = mybir.dt.float32
    bf = mybir.dt.bfloat16
    AF = mybir.ActivationFunctionType
    ALU = mybir.AluOpType

    ERUN = NRB * R
    TPB = R // RBS
    offs = np.concatenate([[0], np.cumsum(Lb)]).astype(np.int64)
    SB = int(offs[2])                  # slots in bond group A = {0, 1}
    TA = SB // RBS                     # full tiles wholly in group A
    GRP = [(0, [0, 1], 0, TA), (1, [2, 3], TA, TPB - TA)]
    TAILLAG = 3
    NG = len(GRP)
    TNMAX = max(TA, TPB - TA)

    WBF, WBFW = _wbf_layout(TPB)

    nc = bacc.Bacc("TRN2", target_bir_lowering=False, debug=False,
                   num_devices=C)

    dspec = [("xembT", (128, ERUN), bf),
             ("maskh", (128, NRB * TPB * RBS), bf),
             ("vge", (128, NRB * 64 * TPB), bf),
             ("b2e", (128, NRB * TPB * H), f32),
             ("w0b0", (128, 256), bf), ("w0r", (128, 768), bf),
             ("w1b0", (128, 256), bf), ("w1r", (128, 768), bf),
             ("bpk", (128, B * 2 * 2 + 1), f32),
             ("wbf", (128, WBFW), bf)]
    dram = {}
    for nm, shp, dt in dspec:
        dram[nm] = nc.dram_tensor(nm, list(shp), dt, kind="ExternalInput").ap()
    outT = nc.dram_tensor("outT", [64, RPC], f32, kind="ExternalOutput").ap()

    with ExitStack() as ctx:
        tc = ctx.enter_context(tile.TileContext(nc))
        constp = ctx.enter_context(tc.tile_pool(name="const", bufs=1))
        hidp = ctx.enter_context(tc.tile_pool(name="hid", bufs=6))
        rhsp = ctx.enter_context(tc.tile_pool(name="rhs", bufs=3))
        ohp = ctx.enter_context(tc.tile_pool(name="oh", bufs=2))
        otp = ctx.enter_context(tc.tile_pool(name="ot", bufs=2))
        rzp = ctx.enter_context(tc.tile_pool(name="rz", bufs=2))
        finp = ctx.enter_context(tc.tile_pool(name="fin", bufs=2))
        # PSUM budget is exactly 8 banks: h0(2) + h1(2) + psA/psP(2) +
        # psE(1) + po(1)
        psh0p = ctx.enter_context(tc.tile_pool(name="psh0", bufs=2,
                                               space="PSUM"))
        psh1p = ctx.enter_context(tc.tile_pool(name="psh1", bufs=2,
                                               space="PSUM"))
        psagp = ctx.enter_context(tc.tile_pool(name="psag", bufs=2,
                                               space="PSUM"))
        pstp = ctx.enter_context(tc.tile_pool(name="pst", bufs=2,
                                              space="PSUM"))

        def _emit_all():
            # DMA order tuned so bond-group-A compute of row-block 0 starts
            # early; everything is resident in SBUF for the whole run
            bpk = constp.tile([128, B * 2 * 2 + 1], f32, tag="bpk", name="bpk")
            nc.sync.dma_start(out=bpk[:], in_=dram["bpk"][:])
            w0b0 = constp.tile([128, 256], bf, tag="w0b0", name="w0b0")
            nc.sync.dma_start(out=w0b0[:], in_=dram["w0b0"][:])
            xeA0 = constp.tile([128, SB], bf, tag="xeA0", name="xeA0")
            nc.sync.dma_start(out=xeA0[:], in_=dram["xembT"][:, 0:SB])
            # trigger the Act function-table load before any data arrives
            dumt = constp.tile([128, 1], f32, tag="dumt", name="dumt")
            nc.vector.memset(dumt[:], 0.0)
            nc.scalar.activation(dumt[:], dumt[:], AF.Exp)
            w1b0 = constp.tile([128, 256], bf, tag="w1b0", name="w1b0")
            nc.sync.dma_start(out=w1b0[:], in_=dram["w1b0"][:])
            w0r = constp.tile([128, 768], bf, tag="w0r", name="w0r")
            nc.sync.dma_start(out=w0r[:], in_=dram["w0r"][:])
            xeB0 = constp.tile([128, R - SB], bf, tag="xeB0", name="xeB0")
            nc.sync.dma_start(out=xeB0[:], in_=dram["xembT"][:, SB:R])
            w1r = constp.tile([128, 768], bf, tag="w1r", name="w1r")
            nc.sync.dma_start(out=w1r[:], in_=dram["w1r"][:])
            wbf = constp.tile([128, WBFW], bf, tag="wbf", name="wbf")
            nc.sync.dma_start(out=wbf[:], in_=dram["wbf"][:])
            masks = []
            t = constp.tile([128, TPB, RBS], bf, tag="mask0", name="mask0")
            nc.sync.dma_start(
                out=t[:], in_=dram["maskh"][:, 0:TPB * RBS]
                .rearrange("p (t r) -> p t r", r=RBS))
            masks.append(t)
            b2esb = constp.tile([128, NRB, H, TPB], f32, tag="b2e",
                                name="b2e")
            nc.sync.dma_start(
                out=b2esb[:],
                in_=dram["b2e"][:].rearrange("p (n h t) -> p n h t",
                                             h=H, t=TPB))
            vges = []
            t = constp.tile([128, 64, TPB], bf, tag="vge0", name="vge0")
            nc.sync.dma_start(
                out=t[:], in_=dram["vge"][:, 0:64 * TPB]
                .rearrange("p (f t) -> p f t", t=TPB))
            vges.append(t)
            # bulk streams for rowblocks 1-3: issue from otherwise-idle
            # engine sequencers so the transfers all enter flight early
            # (a dma_start occupies its issuing sequencer ~0.6us)
            xes = [None]
            dmaeng = [nc.sync, nc.sync, nc.sync]
            for rb in range(1, NRB):
                eng = dmaeng[rb - 1]
                t = constp.tile([128, R], bf, tag=f"xe{rb}", name=f"xe{rb}")
                eng.dma_start(
                    out=t[:], in_=dram["xembT"][:, rb * R:(rb + 1) * R])
                xes.append(t)
                t = constp.tile([128, TPB, RBS], bf, tag=f"mask{rb}",
                                name=f"mask{rb}")
                eng.dma_start(
                    out=t[:],
                    in_=dram["maskh"][:, rb * TPB * RBS:(rb + 1) * TPB * RBS]
                    .rearrange("p (t r) -> p t r", r=RBS))
                masks.append(t)
                t = constp.tile([128, 64, TPB], bf, tag=f"vge{rb}",
                                name=f"vge{rb}")
                eng.dma_start(
                    out=t[:],
                    in_=dram["vge"][:, rb * 64 * TPB:(rb + 1) * 64 * TPB]
                    .rearrange("p (f t) -> p f t", t=TPB))
                vges.append(t)

            def wb(nm):
                o, w = WBF[nm]
                return wbf[:, o:o + w]

            def w0_ap(b):
                return w0b0 if b == 0 else w0r[:, (b - 1) * 256:b * 256]

            def w1_ap(b):
                return w1b0 if b == 0 else w1r[:, (b - 1) * 256:b * 256]

            def xe_ap(rb, lo, ln):
                if rb == 0:
                    if lo < SB:
                        return xeA0[:, lo:lo + ln]
                    return xeB0[:, lo - SB:lo - SB + ln]
                return xes[rb][:, lo:lo + ln]

            # psA ([128, 260] aggregation, one accumulation group per
            # rowblock) shares its bank with psP (projection, opened after
            # the agg group closes). psE (scores) lives in its own bank.
            psAEs = {}
            psEs = {}
            pos = {}
            agg_armed = {}

            # relu engine rotation: GPSIMD cannot read PSUM on real HW,
            # so relus (psum-sourced) go to Act/DVE only; Pool gets the
            # SBUF-side tail work (rhs build, leaky) instead
            _RELUPAT = ["act", "dve"]
            rcnt = [0]

            def _relu(cl, out, in_, bcol):
                eng = _RELUPAT[rcnt[0] % len(_RELUPAT)]
                rcnt[0] += 1
                if eng == "act":
                    nc.scalar.activation(out, in_, AF.Relu, bias=bcol)
                else:
                    e = nc.vector if eng == "dve" else nc.gpsimd
                    e.tensor_scalar(out=out, in0=in_, scalar1=bcol,
                                    scalar2=0.0, op0=ALU.add, op1=ALU.max)

            def ps_views(rb):
                t = psAEs[rb]
                return (t[:, 0:H * 65], t[0:64, 264:264 + 128])

            def emit_head(rb, g, bonds, t0, tn):
                """Per-bond 2-layer MLP; scores land in psE[slot, tile, h].
                Generator: yields after each chunk so the driver can
                interleave pending tail work into the engine streams."""
                if g == 0:
                    psAEs[rb] = psagp.tile([128, 264 + 128], f32,
                                           tag="psa", name=f"psAE{rb}")
                    # psE (scores) and po (transposes) share one bank: both
                    # are written by immediate start/stop matmul groups
                    t = pstp.tile([128, 512], f32, tag="pseo",
                                  name=f"pseo{rb}")
                    psEs[rb] = t[:, 0:TPB * H].rearrange(
                        "p (t h) -> p t h", h=H)
                    pos[rb] = t[0:64, 128:128 + 256].bitcast(bf).rearrange(
                        "p (h t) -> p h t", t=128)
                    agg_armed[rb] = True
                psE = psEs[rb]
                for b in bonds:
                    for (cs, cl) in _chunks(Lb[b]):
                        lo = int(offs[b]) + cs
                        for pr in range(2):
                            i = b * 2 + pr
                            p0 = psh0p.tile([128, 512], f32, tag="h0",
                                            name="p0")
                            nc.tensor.matmul(
                                p0[:, :cl],
                                lhsT=w0_ap(b)[:, pr * 128:(pr + 1) * 128],
                                rhs=xe_ap(rb, lo, cl),
                                start=True, stop=True)
                            h0 = hidp.tile([128, 512], bf, tag="h0s",
                                           name="h0")
                            _relu(cl, h0[:, :cl], p0[:, :cl],
                                  bpk[:, i:i + 1])
                            p1 = psh1p.tile([128, 512], f32, tag="h1",
                                            name="p1")
                            nc.tensor.matmul(
                                p1[:, :cl],
                                lhsT=w1_ap(b)[:, pr * 128:(pr + 1) * 128],
                                rhs=h0[:, :cl],
                                start=True, stop=True)
                            h1 = hidp.tile([128, 512], bf, tag="h1s",
                                           name="h1")
                            _relu(cl, h1[:, :cl], p1[:, :cl],
                                  bpk[:, 8 + i:8 + i + 1])
                            # scores: W2 pieces split at 128-tile boundaries
                            a = lo
                            while a < lo + cl:
                                tb, po_ = a // 128, a % 128
                                # PE psum writes: from partition 32 the span
                                # is capped at 32, so split [32,128) pieces
                                # at the 64 boundary
                                lim = 64 if 32 <= po_ < 64 else 128
                                k = min(lim - po_, lo + cl - a)
                                nc.tensor.matmul(
                                    psE[po_:po_ + k, tb,
                                        pr * 2:pr * 2 + 2],
                                    lhsT=h1[:, a - lo:a - lo + k],
                                    rhs=wb("w2all")[:, i * 2:(i + 1) * 2],
                                    start=True, stop=True)
                                a += k
                            yield

            def emit_tail(rb, g, bonds, t0, tn):
                psA, psP = ps_views(rb)
                psE = psEs[rb]
                last = (g == NG - 1)
                # rhs[p, h, 0:64, t] = vge[p, :, t] * w[p, h, t];
                # rhs[p, h, 64, t] = w[p, h, t] = exp(leaky(psE + b2e)),
                # computed in place in the weight column, so one 260-col
                # matmul per tile aggregates values + normalizer together.
                # Tiles are processed in two batches to halve tail latency.
                rhs = rhsp.tile([128, H, 65, TNMAX], bf, tag="rhs",
                                name="rhs")
                hq = (tn + 1) // 2
                wcol = rhs[:, :, 64, 0:tn]
                nc.vector.tensor_tensor(
                    out=wcol,
                    in0=psE[:, t0:t0 + tn, :].rearrange("p t h -> p h t"),
                    in1=b2esb[:, rb, :, t0:t0 + tn], op=ALU.add)
                nc.vector.scalar_tensor_tensor(
                    out=wcol, in0=wcol, scalar=NEG,
                    op0=ALU.mult, in1=wcol, op1=ALU.max)
                nc.scalar.activation(wcol, wcol, AF.Exp)
                yield
                for (q0, qn) in ((0, hq), (hq, tn - hq)):
                    if qn <= 0:
                        continue
                    tq = t0 + q0
                    nc.gpsimd.tensor_tensor(
                        out=rhs[:, 0:4, 0:64, q0:q0 + qn],
                        in0=vges[rb][:, :, tq:tq + qn].unsqueeze(1)
                        .to_broadcast([128, 4, 64, qn]),
                        in1=rhs[:, 0:4, 64:65, q0:q0 + qn]
                        .to_broadcast([128, 4, 64, qn]),
                        op=ALU.mult)
                    # segment softmax-aggregate: ONE psum accumulation group
                    # per rowblock (start on first matmul, stop on the last)
                    for q in range(q0, q0 + qn):
                        st = agg_armed[rb]
                        agg_armed[rb] = False
                        nc.tensor.matmul(psA[:],
                                         lhsT=masks[rb][:, t0 + q, :],
                                         rhs=rhs[:, :, :, q],
                                         start=st,
                                         stop=last and q == tn - 1)
                    yield
                if not last:
                    return

                # normalize -> transpose -> project -> ship out
                rz = rzp.tile([128, H], f32, tag="rz", name="rz")
                nc.vector.reciprocal(
                    rz[:], psA[:].rearrange("p (h z) -> p h z", z=65)[:, :, 64])
                ohsb = ohp.tile([128, H, 64], bf, tag="oh", name="oh")
                for h in range(H):
                    if h % 2 == 0:
                        nc.vector.tensor_tensor(
                            out=ohsb[:, h, :],
                            in0=psA[:, h * 65:h * 65 + 64],
                            in1=rz[:, h:h + 1].to_broadcast([128, 64]),
                            op=ALU.mult)
                    else:
                        nc.scalar.activation(
                            ohsb[:, h, :], psA[:, h * 65:h * 65 + 64],
                            AF.Identity, scale=rz[:, h:h + 1])
                yield
                po = pos[rb]
                for h in range(H):
                    nc.tensor.transpose(out=po[:, h, :], in_=ohsb[:, h, :],
                                        identity=wb("id128"))
                otrb = otp.tile([64, H, 128], bf, tag="ot", name="ot")
                nc.vector.tensor_copy(otrb[:], po[:])
                for h in range(H):
                    nc.tensor.matmul(
                        psP[:],
                        lhsT=wb("pw4")[0:64, h * 64:(h + 1) * 64],
                        rhs=otrb[:, h, :],
                        start=(h == 0), stop=(h == H - 1))
                outsb = finp.tile([64, 128], f32, tag="outsb", name="outsb")
                nc.scalar.activation(outsb[:], psP[:], AF.Identity,
                                     bias=bpk[0:64, 16:17])
                nc.sync.dma_start(out=outT[:, rb * 128:(rb + 1) * 128],
                                  in_=outsb[:])

            # software pipeline: head and tail are generators; tail steps of
            # older units are interleaved between the MLP chunks of newer
            # units so aggregation work never bunches up on any engine
            units = [(rb, *grp) for rb in range(NRB) for grp in GRP]
            import collections as _c
            tails = _c.deque()

            def pump_tail():
                while tails:
                    try:
                        next(tails[0])
                        return
                    except StopIteration:
                        tails.popleft()

            for u in units:
                for _ in emit_head(*u):
                    if len(tails) >= TAILLAG:
                        pump_tail()
                tails.append(emit_tail(*u))
            while tails:
                pump_tail()

        if loop:
            with tc.For_i(0, loop, 1):
                _emit_all()
        else:
            _emit_all()

    nc.compile()
    return nc


def _prepare(inputs):
    import ml_dtypes
    bf16 = ml_dtypes.bfloat16
    hp = _host_prep(inputs["embeddings"], inputs["src"], inputs["dst"],
                    inputs["bond"])
    wts = _weights_prep(inputs)
    Lb, R, TPB = hp["Lb"], hp["R"], hp["TPB"]
    f32 = np.float32

    # b2slot [C,128,NRB,TPB,H] holds bond ids; b2 index = (bond, head);
    # shipped head-major as [128, NRB, H, TPB]
    b2e = wts["b2"][hp["b2slot"], np.arange(H)[None, None, None, None, :]]
    b2e = np.ascontiguousarray(b2e.transpose(0, 1, 2, 4, 3))

    bpk = np.zeros((128, B * 2 * 2 + 1), f32)
    bpk[:, 0:8] = wts["b0all"]
    bpk[:, 8:16] = wts["b1all"]
    bpk[0:64, 16:17] = wts["biascol"]

    WBF, WBFW = _wbf_layout(TPB)
    wbf = np.zeros((128, WBFW), bf16)
    o, w = WBF["w2all"]
    wbf[:, o:o + w] = wts["w2all"].astype(bf16)
    o, w = WBF["pw4"]
    wbf[0:64, o:o + w] = wts["pw4"].astype(bf16)
    o, w = WBF["id128"]
    wbf[:, o:o + w] = wts["id128"].astype(bf16)

    w0b0 = np.ascontiguousarray(wts["w0all"][:, 0:256]).astype(bf16)
    w0r = np.ascontiguousarray(wts["w0all"][:, 256:1024]).astype(bf16)
    w1b0 = np.ascontiguousarray(wts["w1all"][:, 0:256]).astype(bf16)
    w1r = np.ascontiguousarray(wts["w1all"][:, 256:1024]).astype(bf16)

    key = (Lb, R)
    if key not in _cache:
        _cache.clear()
        _cache[key] = _build_program(Lb, R)
    nc = _cache[key]
    in_maps = []
    for c in range(C):
        m = {"xembT": hp["xembT"][c].astype(bf16),
             "maskh": hp["maskh"][c].reshape(128, -1).astype(bf16),
             "vge": hp["vge"][c].reshape(128, -1).astype(bf16),
             "b2e": b2e[c].reshape(128, -1).astype(f32),
             "w0b0": w0b0, "w0r": w0r, "w1b0": w1b0, "w1r": w1r,
             "bpk": bpk, "wbf": wbf}
        in_maps.append(m)
    return nc, in_maps, hp


def kernel(**inputs):
    from concourse.bass_utils import run_bass_kernel_spmd

    nc, in_maps, hp = _prepare(inputs)
    res = run_bass_kernel_spmd(nc, in_maps, list(range(C)))
    out = np.empty((N, D), np.float32)
    rowmap = hp["rowmap"].reshape(C, RPC)
    for c in range(C):
        out[rowmap[c]] = res.results[c]["outT"].T
    return out


def benchmark(inputs, iters=10, warmup=2):
    """Time repeated executions of the compiled SPMD program with
    device-resident inputs (excludes compile and host<->device transfer)."""
    import time
    import jax
    from jax.experimental.shard_map import shard_map
    from jax.sharding import Mesh, PartitionSpec, NamedSharding
    from concourse import bass2jax as b2j
    from concourse import mybir

    nc, in_maps, _hp = _prepare(inputs)
    b2j.install_neuronx_cc_hook()
    partition_name = (nc.partition_id_tensor.name
                      if nc.partition_id_tensor else None)
    in_names, out_names, out_avals, zero_outs = [], [], [], []
    for alloc in nc.m.functions[0].allocations:
        if not isinstance(alloc, mybir.MemoryLocationSet):
            continue
        name = alloc.memorylocations[0].name
        if alloc.kind == "ExternalInput":
            if name != partition_name:
                in_names.append(name)
        elif alloc.kind == "ExternalOutput":
            out_names.append(name)
            shape = tuple(alloc.tensor_shape)
            dtype = mybir.dt.np(alloc.dtype)
            out_avals.append(jax.core.ShapedArray(shape, dtype))
            zero_outs.append(np.zeros(shape, dtype))
    n_params = len(in_names)
    all_in = in_names + out_names + ([partition_name] if partition_name
                                     else [])
    donate = tuple(range(n_params, n_params + len(out_names)))

    def _body(*args):
        operands = list(args)
        if partition_name is not None:
            operands.append(b2j.partition_id_tensor())
        outs = b2j._bass_exec_p.bind(
            *operands, out_avals=tuple(out_avals), in_names=tuple(all_in),
            out_names=tuple(out_names), lowering_input_output_aliases=(),
            sim_require_finite=True, sim_require_nnan=True, nc=nc)
        return tuple(outs)

    devices = jax.devices()[:C]
    mesh = Mesh(np.asarray(devices), ("core",))
    in_specs = (PartitionSpec("core"),) * (n_params + len(out_names))
    out_specs = (PartitionSpec("core"),) * len(out_names)
    sharded = jax.jit(shard_map(_body, mesh=mesh, in_specs=in_specs,
                                out_specs=out_specs, check_rep=False),
                      donate_argnums=donate, keep_unused=True)
    sh = NamedSharding(mesh, PartitionSpec("core"))
    concat_in = [
        jax.device_put(
            np.concatenate([np.asarray(in_maps[c][n]) for c in range(C)],
                           axis=0), sh)
        for n in in_names]

    times = []
    for it in range(warmup + iters):
        zs = [jax.device_put(np.zeros((C * z.shape[0], *z.shape[1:]),
                                      z.dtype), sh)
              for z in zero_outs]
        t0 = time.perf_counter()
        out = sharded(*concat_in, *zs)
        jax.block_until_ready(out)
        dt = time.perf_counter() - t0
        if it >= warmup:
            times.append(dt)
    print("bench times (ms):", [f"{t*1e3:.3f}" for t in times])
    return min(times) * 1e9


def benchmark_hw(inputs, k=512, iters=6, warmup=2, k_small=None):
    """Real-HW timing: run the whole per-core program k times inside one
    NEFF (tc.For_i) and wall-time it through the tunnel. If k_small is
    given, also times a k_small-loop NEFF and returns the difference
    quotient, which cancels the (~80ms) tunnel dispatch floor exactly."""
    if k_small:
        t_big = benchmark_hw(inputs, k=k, iters=iters, warmup=warmup)
        t_sml = benchmark_hw(inputs, k=k_small, iters=iters, warmup=warmup)
        return (t_big * k - t_sml * k_small) / (k - k_small)
    import time
    import jax
    from jax.experimental.shard_map import shard_map
    from jax.sharding import Mesh, PartitionSpec, NamedSharding
    from concourse import bass2jax as b2j
    from concourse import mybir

    nc0, in_maps, hp = _prepare(inputs)
    nc = _build_program(hp["Lb"], hp["R"], loop=k)

    b2j.install_neuronx_cc_hook()
    partition_name = (nc.partition_id_tensor.name
                      if nc.partition_id_tensor else None)
    in_names, out_names, out_avals, zero_outs = [], [], [], []
    for alloc in nc.m.functions[0].allocations:
        if not isinstance(alloc, mybir.MemoryLocationSet):
            continue
        name = alloc.memorylocations[0].name
        if alloc.kind == "ExternalInput":
            if name != partition_name:
                in_names.append(name)
        elif alloc.kind == "ExternalOutput":
            out_names.append(name)
            shape = tuple(alloc.tensor_shape)
            dtype = mybir.dt.np(alloc.dtype)
            out_avals.append(jax.core.ShapedArray(shape, dtype))
            zero_outs.append(np.zeros(shape, dtype))
    n_params = len(in_names)
    all_in = in_names + out_names + ([partition_name] if partition_name
                                     else [])
    donate = tuple(range(n_params, n_params + len(out_names)))

    def _body(*args):
        operands = list(args)
        if partition_name is not None:
            operands.append(b2j.partition_id_tensor())
        outs = b2j._bass_exec_p.bind(
            *operands, out_avals=tuple(out_avals), in_names=tuple(all_in),
            out_names=tuple(out_names), lowering_input_output_aliases=(),
            sim_require_finite=True, sim_require_nnan=True, nc=nc)
        return tuple(outs)

    devices = jax.devices()[:C]
    mesh = Mesh(np.asarray(devices), ("core",))
    in_specs = (PartitionSpec("core"),) * (n_params + len(out_names))
    out_specs = (PartitionSpec("core"),) * len(out_names)
    sharded = jax.jit(shard_map(_body, mesh=mesh, in_specs=in_specs,
                                out_specs=out_specs, check_rep=False),
                      donate_argnums=donate, keep_unused=True)
    sh = NamedSharding(mesh, PartitionSpec("core"))
    concat_in = [
        jax.device_put(
            np.concatenate([np.asarray(in_maps[c][n]) for c in range(C)],
                           axis=0), sh)
        for n in in_names]
    times = []
    for it in range(warmup + iters):
        zs = [jax.device_put(np.zeros((C * z.shape[0], *z.shape[1:]),
                                      z.dtype), sh)
              for z in zero_outs]
        t0 = time.perf_counter()
        out = sharded(*concat_in, *zs)
        jax.block_until_ready(out)
        dt = time.perf_counter() - t0
        if it >= warmup:
            times.append(dt)
    print("looped bench times (ms):", [f"{t*1e3:.2f}" for t in times])
    best = min(times)
    return best * 1e9 / k


# revision 69
# speedup vs baseline: 1.1869x; 1.0517x over previous
"""GeAT layer (graph attention w/ per-edge MLP scoring) on 8 Trainium2 cores.

Strategy (fully sparse — the dense [H,N,N] tensor is never materialized):
  - Directed edges (symmetric doubling) are deduped (last-write-wins, matching
    XLA scatter-set) and rows are PERMUTED into 32 balanced buckets of 128
    (greedy load balancing on per-row per-bond edge counts), so every
    (core, rowblock, bond) slot range has the same near-minimal capacity.
    Fully data-parallel: no cross-core communication.
  - Host prep ships, per core: the edge stream feat-major ([128, E] = Qs/Kd
    halves), the dst-embedding stream edge-major (for the aggregation rhs),
    the 0/1 scatter mask (slot -> row-in-block), and the per-slot b2 bias.
    Qw/Kw/Qb/Kb are fused into the first MLP layer; Vw/Vb are fused into the
    output projection (aggregation runs on raw dst embeddings).
  - On device, per (rowblock, bondgroup) unit: per-bond 2-layer MLP (heads
    packed in pairs of 64) -> scores -> leaky/exp -> weight-scaled dst-emb
    rhs -> PSUM-accumulated mask matmuls (segment softmax-aggregate) ->
    normalize -> transpose -> fused projection. Elementwise work is spread
    across Act/DVE/Pool so the tensor engine is the critical resource.
"""

import sys

sys.path.insert(0, "/opt/trn_rl_repo")

import numpy as np

N, D, H, B, HID = 4096, 64, 4, 4, 64
NEG = 0.2
C = 8            # cores
RPC = N // C     # rows per core
NRB = 4          # row blocks per core
RBS = 128        # rows per block
NBUCK = C * NRB  # 32 row buckets

_cache = {}


def _balance_rows(cnt):
    """Assign 4096 rows to 32 buckets of 128, minimizing per-bond max."""
    nb = NBUCK
    target = cnt.sum(0) / nb
    order = np.argsort(-cnt.sum(1), kind="stable")
    sums = np.zeros((nb, B))
    sizes = np.zeros(nb, np.int64)
    assign = np.empty(N, np.int64)
    for r in order:
        cand = np.where(sizes < RBS)[0]
        news = sums[cand] + cnt[r]
        cost = (news / target).max(1) + 0.0005 * sizes[cand]
        k = cand[int(np.argmin(cost))]
        assign[r] = k
        sums[k] += cnt[r]
        sizes[k] += 1
    maxes = sums.max(0).astype(np.int64)
    # per-bond caps: round achieved maxes up to 32, pad the total to a
    # multiple of 128 (tile size), and keep every bond-start offset at
    # 0/32/64 mod 128 (PE psum writes can't start at partition 96)
    base = [int(-(-m // 32) * 32) for m in maxes]
    best = None
    import itertools
    for pads in itertools.product((0, 32, 64, 96, 128), repeat=B):
        Lb = [base[b] + pads[b] for b in range(B)]
        if sum(Lb) % RBS != 0:
            continue
        if any(sum(Lb[:b]) % RBS == 96 for b in range(1, B)):
            continue
        if best is None or sum(Lb) < sum(best):
            best = Lb
    return assign, best


def _host_prep(embeddings, src, dst, bond):
    emb = np.ascontiguousarray(np.asarray(embeddings, np.float32))
    src = np.asarray(src).astype(np.int64)
    dst = np.asarray(dst).astype(np.int64)
    bond = np.asarray(bond).astype(np.int64)

    s_all = np.concatenate([src, dst])
    d_all = np.concatenate([dst, src])
    b_all = np.concatenate([bond, bond])
    L = s_all.shape[0]

    # scatter-set duplicate resolution: last occurrence wins
    key = s_all * N + d_all
    order = np.argsort(key, kind="stable")
    ks = key[order]
    is_last = np.ones(L, bool)
    is_last[:-1] = ks[1:] != ks[:-1]
    alive = np.zeros(L, bool)
    alive[order[is_last]] = True

    cnt = np.zeros((N, B), np.int64)
    np.add.at(cnt, (s_all[alive], b_all[alive]), 1)
    assign, Lb = _balance_rows(cnt)

    R = int(sum(Lb))
    TPB = R // RBS
    NTILE = NRB * TPB
    ERUN = NRB * R
    offs = np.concatenate([[0], np.cumsum(Lb)]).astype(np.int64)

    # rows of bucket k in stable order; srel = index within bucket
    rowmap = np.empty((NBUCK, RBS), np.int64)
    srel_of = np.empty(N, np.int64)
    for k in range(NBUCK):
        rows = np.where(assign == k)[0]
        rowmap[k] = rows
        srel_of[rows] = np.arange(RBS)

    core_of = assign // NRB
    rb_of = assign % NRB

    ecore = core_of[s_all]
    erb = rb_of[s_all]
    esrel = srel_of[s_all]

    xembT = np.zeros((C, 128, ERUN), np.float32)
    maskh = np.zeros((C, 128, NRB, TPB * RBS), np.float32)
    vge = np.zeros((C, 128, NRB, 64, TPB), np.float32)
    b2slot = np.zeros((C, 128, NRB, TPB, H), np.int64)  # bond id per slot
    for c in range(C):
        for r in range(NRB):
            for b in range(B):
                sel = np.where(alive & (ecore == c) & (erb == r)
                               & (b_all == b))[0]
                lo = int(offs[b])
                allslots = lo + np.arange(Lb[b])
                b2slot[c, allslots % 128, r, allslots // 128, :] = b
                if len(sel) == 0:
                    continue
                slots = lo + np.arange(len(sel))
                p, t = slots % 128, slots // 128
                gs = r * R + slots
                xembT[c, 0:64, gs] = emb[s_all[sel]]
                xembT[c, 64:128, gs] = emb[d_all[sel]]
                maskh[c, p, r, t * RBS + esrel[sel]] = 1.0
                vge[c, p, r, :, t] = emb[d_all[sel]]
    return dict(xembT=xembT, maskh=maskh, vge=vge, b2slot=b2slot,
                rowmap=rowmap, Lb=tuple(int(x) for x in Lb), R=R,
                TPB=TPB, NTILE=NTILE, ERUN=ERUN)


def _weights_prep(inp):
    f32 = np.float32
    Qw, Qb = np.asarray(inp["Qw"], f32), np.asarray(inp["Qb"], f32)
    Kw, Kb = np.asarray(inp["Kw"], f32), np.asarray(inp["Kb"], f32)
    Vw, Vb = np.asarray(inp["Vw"], f32), np.asarray(inp["Vb"], f32)
    W0, b0 = np.asarray(inp["W0"], f32), np.asarray(inp["b0"], f32)
    W1, b1 = np.asarray(inp["W1"], f32), np.asarray(inp["b1"], f32)
    W2, b2 = np.asarray(inp["W2"], f32), np.asarray(inp["b2"], f32)
    Pw, Pb = np.asarray(inp["Pw"], f32), np.asarray(inp["Pb"], f32)

    # fuse the Q/K projections into the first MLP layer (per bond, head)
    fw0 = np.zeros((B, H, 128, HID), f32)
    fb0 = np.zeros((B, H, HID), f32)
    for b in range(B):
        for h in range(H):
            fw0[b, h, 0:64] = Qw @ W0[b, h, 0:64]
            fw0[b, h, 64:128] = Kw @ W0[b, h, 64:128]
            fb0[b, h] = Qb @ W0[b, h, 0:64] + Kb @ W0[b, h, 64:128] + b0[b, h]

    w0all = np.zeros((128, B * 2 * 128), f32)
    w1all = np.zeros((128, B * 2 * 128), f32)
    w2all = np.zeros((128, B * 2 * 2), f32)
    b0all = np.zeros((128, B * 2), f32)
    b1all = np.zeros((128, B * 2), f32)
    for b in range(B):
        for pr in range(2):
            i = b * 2 + pr
            ha, hb = 2 * pr, 2 * pr + 1
            w0all[:, i * 128: i * 128 + 64] = fw0[b, ha]
            w0all[:, i * 128 + 64: (i + 1) * 128] = fw0[b, hb]
            w1all[0:64, i * 128: i * 128 + 64] = W1[b, ha]
            w1all[64:128, i * 128 + 64: (i + 1) * 128] = W1[b, hb]
            w2all[0:64, i * 2] = W2[b, ha]
            w2all[64:128, i * 2 + 1] = W2[b, hb]
            b0all[0:64, i] = fb0[b, ha]
            b0all[64:128, i] = fb0[b, hb]
            b1all[0:64, i] = b1[b, ha]
            b1all[64:128, i] = b1[b, hb]

    # fold Vw into the projection: out = concat_h(aggemb_h) @ (Vw @ Pw_h) + bias
    pw4 = np.zeros((64, H * 64), f32)
    for h in range(H):
        pw4[:, h * 64:(h + 1) * 64] = Vw @ Pw[h * 64:(h + 1) * 64]
    biascol = (Pb + np.tile(Vb, H) @ Pw)[:, None]         # [64, 1]
    id128 = np.eye(128, dtype=f32)

    return dict(w0all=w0all, w1all=w1all, w2all=w2all,
                b0all=b0all, b1all=b1all, b2=b2,
                pw4=pw4, biascol=biascol, id128=id128)


def _chunks(n, step=512):
    out = []
    s = 0
    while s < n:
        out.append((s, min(step, n - s)))
        s += step
    return out


# packed constant column layouts
def _wbf_layout(TPB):
    WBF = {}
    o = 0
    for nm, w in [("w2all", B * 2 * 2), ("pw4", H * 64), ("id128", 128)]:
        WBF[nm] = (o, w)
        o += w
    return WBF, o


def _build_program(Lb, R, loop=0):
    import concourse.bacc as bacc
    import concourse.tile as tile
    from concourse import mybir
    from contextlib import ExitStack

    f32 = mybir.dt.float32
    bf = mybir.dt.bfloat16
    AF = mybir.ActivationFunctionType
    ALU = mybir.AluOpType

    ERUN = NRB * R
    TPB = R // RBS
    offs = np.concatenate([[0], np.cumsum(Lb)]).astype(np.int64)
    SB = int(offs[2])                  # slots in bond group A = {0, 1}
    TA = SB // RBS                     # full tiles wholly in group A
    GRP = [(0, [0, 1], 0, TA), (1, [2, 3], TA, TPB - TA)]
    TAILLAG = 4
    NG = len(GRP)
    TNMAX = max(TA, TPB - TA)

    WBF, WBFW = _wbf_layout(TPB)

    nc = bacc.Bacc("TRN2", target_bir_lowering=False, debug=False,
                   num_devices=C)

    SEG = R + TPB * RBS + 64 * TPB     # per-rowblock packed stream cols
    dspec = [("strm", (128, NRB * SEG), bf),
             ("b2e", (128, NRB * TPB * H), f32),
             ("w0b0", (128, 256), bf), ("w0r", (128, 768), bf),
             ("w1b0", (128, 256), bf), ("w1r", (128, 768), bf),
             ("bpk", (128, B * 2 * 2 + 1), f32),
             ("wbf", (128, WBFW), bf)]
    dram = {}
    for nm, shp, dt in dspec:
        dram[nm] = nc.dram_tensor(nm, list(shp), dt, kind="ExternalInput").ap()
    outT = nc.dram_tensor("outT", [64, RPC], f32, kind="ExternalOutput").ap()

    with ExitStack() as ctx:
        tc = ctx.enter_context(tile.TileContext(nc))
        constp = ctx.enter_context(tc.tile_pool(name="const", bufs=1))
        hidp = ctx.enter_context(tc.tile_pool(name="hid", bufs=6))
        rhsp = ctx.enter_context(tc.tile_pool(name="rhs", bufs=3))
        ohp = ctx.enter_context(tc.tile_pool(name="oh", bufs=2))
        otp = ctx.enter_context(tc.tile_pool(name="ot", bufs=2))
        rzp = ctx.enter_context(tc.tile_pool(name="rz", bufs=2))
        finp = ctx.enter_context(tc.tile_pool(name="fin", bufs=2))
        # PSUM budget is exactly 8 banks: h0(2) + h1(2) + psA/psP(2) +
        # psE(1) + po(1)
        psh0p = ctx.enter_context(tc.tile_pool(name="psh0", bufs=2,
                                               space="PSUM"))
        psh1p = ctx.enter_context(tc.tile_pool(name="psh1", bufs=2,
                                               space="PSUM"))
        psagp = ctx.enter_context(tc.tile_pool(name="psag", bufs=2,
                                               space="PSUM"))
        pstp = ctx.enter_context(tc.tile_pool(name="pst", bufs=2,
                                              space="PSUM"))

        def _emit_all():
            # DMA order tuned so bond-group-A compute of row-block 0 starts
            # early; everything is resident in SBUF for the whole run
            bpk = constp.tile([128, B * 2 * 2 + 1], f32, tag="bpk", name="bpk")
            nc.sync.dma_start(out=bpk[:], in_=dram["bpk"][:])
            w0b0 = constp.tile([128, 256], bf, tag="w0b0", name="w0b0")
            nc.sync.dma_start(out=w0b0[:], in_=dram["w0b0"][:])
            # per-rowblock packed stream tiles [xe | mask | vge]; rb0's
            # pieces are fetched separately so compute starts early
            strms = []
            for rb in range(NRB):
                strms.append(constp.tile([128, SEG], bf, tag=f"strm{rb}",
                                         name=f"strm{rb}"))
            nc.sync.dma_start(out=strms[0][:, 0:SB],
                              in_=dram["strm"][:, 0:SB])
            # trigger the Act function-table load before any data arrives
            dumt = constp.tile([128, 1], f32, tag="dumt", name="dumt")
            nc.vector.memset(dumt[:], 0.0)
            nc.scalar.activation(dumt[:], dumt[:], AF.Exp)
            w1b0 = constp.tile([128, 256], bf, tag="w1b0", name="w1b0")
            nc.sync.dma_start(out=w1b0[:], in_=dram["w1b0"][:])
            w0r = constp.tile([128, 768], bf, tag="w0r", name="w0r")
            nc.sync.dma_start(out=w0r[:], in_=dram["w0r"][:])
            nc.sync.dma_start(out=strms[0][:, SB:R],
                              in_=dram["strm"][:, SB:R])
            w1r = constp.tile([128, 768], bf, tag="w1r", name="w1r")
            nc.sync.dma_start(out=w1r[:], in_=dram["w1r"][:])
            wbf = constp.tile([128, WBFW], bf, tag="wbf", name="wbf")
            nc.sync.dma_start(out=wbf[:], in_=dram["wbf"][:])
            nc.sync.dma_start(out=strms[0][:, R:SEG],
                              in_=dram["strm"][:, R:SEG])
            b2esb = constp.tile([128, NRB, H, TPB], f32, tag="b2e",
                                name="b2e")
            nc.sync.dma_start(
                out=b2esb[:],
                in_=dram["b2e"][:].rearrange("p (n h t) -> p n h t",
                                             h=H, t=TPB))
            for rb in range(1, NRB):
                nc.sync.dma_start(
                    out=strms[rb][:, 0:R],
                    in_=dram["strm"][:, rb * SEG:rb * SEG + R])
                nc.sync.dma_start(
                    out=strms[rb][:, R:SEG],
                    in_=dram["strm"][:, rb * SEG + R:(rb + 1) * SEG])
            xes = [strms[rb][:, 0:R] for rb in range(NRB)]
            masks = [strms[rb][:, R:R + TPB * RBS]
                     .rearrange("p (t r) -> p t r", r=RBS)
                     for rb in range(NRB)]
            vges = [strms[rb][:, R + TPB * RBS:SEG]
                    .rearrange("p (f t) -> p f t", t=TPB)
                    for rb in range(NRB)]

            def wb(nm):
                o, w = WBF[nm]
                return wbf[:, o:o + w]

            def w0_ap(b):
                return w0b0 if b == 0 else w0r[:, (b - 1) * 256:b * 256]

            def w1_ap(b):
                return w1b0 if b == 0 else w1r[:, (b - 1) * 256:b * 256]

            def xe_ap(rb, lo, ln):
                return xes[rb][:, lo:lo + ln]

            # psA ([128, 260] aggregation, one accumulation group per
            # rowblock) shares its bank with psP (projection, opened after
            # the agg group closes). psE (scores) lives in its own bank.
            psAEs = {}
            psEs = {}
            pos = {}
            agg_armed = {}

            # relu engine rotation: GPSIMD cannot read PSUM on real HW,
            # so relus (psum-sourced) go to Act/DVE only; Pool gets the
            # SBUF-side tail work (rhs build, leaky) instead
            _RELUPAT = ["act", "dve"]
            rcnt = [0]

            def _relu(cl, out, in_, bcol):
                eng = _RELUPAT[rcnt[0] % len(_RELUPAT)]
                rcnt[0] += 1
                if eng == "act":
                    nc.scalar.activation(out, in_, AF.Relu, bias=bcol)
                else:
                    e = nc.vector if eng == "dve" else nc.gpsimd
                    e.tensor_scalar(out=out, in0=in_, scalar1=bcol,
                                    scalar2=0.0, op0=ALU.add, op1=ALU.max)

            def ps_views(rb):
                t = psAEs[rb]
                return (t[:, 0:H * 65], t[0:64, 264:264 + 128])

            def emit_head(rb, g, bonds, t0, tn):
                """Per-bond 2-layer MLP; scores land in psE[slot, tile, h].
                Generator: yields after each chunk so the driver can
                interleave pending tail work into the engine streams."""
                if g == 0:
                    psAEs[rb] = psagp.tile([128, 264 + 128], f32,
                                           tag="psa", name=f"psAE{rb}")
                    # psE (scores) and po (transposes) share one bank: both
                    # are written by immediate start/stop matmul groups
                    t = pstp.tile([128, 512], f32, tag="pseo",
                                  name=f"pseo{rb}")
                    psEs[rb] = t[:, 0:TPB * H].rearrange(
                        "p (t h) -> p t h", h=H)
                    pos[rb] = t[0:64, 128:128 + 256].bitcast(bf).rearrange(
                        "p (h t) -> p h t", t=128)
                    agg_armed[rb] = True
                psE = psEs[rb]
                for b in bonds:
                    for (cs, cl) in _chunks(Lb[b]):
                        lo = int(offs[b]) + cs
                        for pr in range(2):
                            i = b * 2 + pr
                            p0 = psh0p.tile([128, 512], f32, tag="h0",
                                            name="p0")
                            nc.tensor.matmul(
                                p0[:, :cl],
                                lhsT=w0_ap(b)[:, pr * 128:(pr + 1) * 128],
                                rhs=xe_ap(rb, lo, cl),
                                start=True, stop=True)
                            h0 = hidp.tile([128, 512], bf, tag="h0s",
                                           name="h0")
                            _relu(cl, h0[:, :cl], p0[:, :cl],
                                  bpk[:, i:i + 1])
                            p1 = psh1p.tile([128, 512], f32, tag="h1",
                                            name="p1")
                            nc.tensor.matmul(
                                p1[:, :cl],
                                lhsT=w1_ap(b)[:, pr * 128:(pr + 1) * 128],
                                rhs=h0[:, :cl],
                                start=True, stop=True)
                            h1 = hidp.tile([128, 512], bf, tag="h1s",
                                           name="h1")
                            _relu(cl, h1[:, :cl], p1[:, :cl],
                                  bpk[:, 8 + i:8 + i + 1])
                            # scores: W2 pieces split at 128-tile boundaries
                            a = lo
                            while a < lo + cl:
                                tb, po_ = a // 128, a % 128
                                # PE psum writes: from partition 32 the span
                                # is capped at 32, so split [32,128) pieces
                                # at the 64 boundary
                                lim = 64 if 32 <= po_ < 64 else 128
                                k = min(lim - po_, lo + cl - a)
                                nc.tensor.matmul(
                                    psE[po_:po_ + k, tb,
                                        pr * 2:pr * 2 + 2],
                                    lhsT=h1[:, a - lo:a - lo + k],
                                    rhs=wb("w2all")[:, i * 2:(i + 1) * 2],
                                    start=True, stop=True)
                                a += k
                            yield

            def emit_tail(rb, g, bonds, t0, tn):
                psA, psP = ps_views(rb)
                psE = psEs[rb]
                last = (g == NG - 1)
                # rhs[p, h, 0:64, t] = vge[p, :, t] * w[p, h, t];
                # rhs[p, h, 64, t] = w[p, h, t] = exp(leaky(psE + b2e)),
                # computed in place in the weight column, so one 260-col
                # matmul per tile aggregates values + normalizer together.
                # Tiles are processed in two batches to halve tail latency.
                rhs = rhsp.tile([128, H, 65, TNMAX], bf, tag="rhs",
                                name="rhs")
                hq = (tn + 1) // 2
                wcol = rhs[:, :, 64, 0:tn]
                nc.vector.tensor_tensor(
                    out=wcol,
                    in0=psE[:, t0:t0 + tn, :].rearrange("p t h -> p h t"),
                    in1=b2esb[:, rb, :, t0:t0 + tn], op=ALU.add)
                nc.vector.scalar_tensor_tensor(
                    out=wcol, in0=wcol, scalar=NEG,
                    op0=ALU.mult, in1=wcol, op1=ALU.max)
                nc.scalar.activation(wcol, wcol, AF.Exp)
                yield
                for (q0, qn) in ((0, hq), (hq, tn - hq)):
                    if qn <= 0:
                        continue
                    tq = t0 + q0
                    nc.gpsimd.tensor_tensor(
                        out=rhs[:, 0:4, 0:64, q0:q0 + qn],
                        in0=vges[rb][:, :, tq:tq + qn].unsqueeze(1)
                        .to_broadcast([128, 4, 64, qn]),
                        in1=rhs[:, 0:4, 64:65, q0:q0 + qn]
                        .to_broadcast([128, 4, 64, qn]),
                        op=ALU.mult)
                    # segment softmax-aggregate: ONE psum accumulation group
                    # per rowblock (start on first matmul, stop on the last)
                    for q in range(q0, q0 + qn):
                        st = agg_armed[rb]
                        agg_armed[rb] = False
                        nc.tensor.matmul(psA[:],
                                         lhsT=masks[rb][:, t0 + q, :],
                                         rhs=rhs[:, :, :, q],
                                         start=st,
                                         stop=last and q == tn - 1)
                    yield
                if not last:
                    return

                # normalize -> transpose -> project -> ship out
                rz = rzp.tile([128, H], f32, tag="rz", name="rz")
                nc.vector.reciprocal(
                    rz[:], psA[:].rearrange("p (h z) -> p h z", z=65)[:, :, 64])
                ohsb = ohp.tile([128, H, 64], bf, tag="oh", name="oh")
                for h in range(H):
                    if h % 2 == 0:
                        nc.vector.tensor_tensor(
                            out=ohsb[:, h, :],
                            in0=psA[:, h * 65:h * 65 + 64],
                            in1=rz[:, h:h + 1].to_broadcast([128, 64]),
                            op=ALU.mult)
                    else:
                        nc.scalar.activation(
                            ohsb[:, h, :], psA[:, h * 65:h * 65 + 64],
                            AF.Identity, scale=rz[:, h:h + 1])
                yield
                po = pos[rb]
                for h in range(H):
                    nc.tensor.transpose(out=po[:, h, :], in_=ohsb[:, h, :],
                                        identity=wb("id128"))
                otrb = otp.tile([64, H, 128], bf, tag="ot", name="ot")
                nc.vector.tensor_copy(otrb[:], po[:])
                for h in range(H):
                    nc.tensor.matmul(
                        psP[:],
                        lhsT=wb("pw4")[0:64, h * 64:(h + 1) * 64],
                        rhs=otrb[:, h, :],
                        start=(h == 0), stop=(h == H - 1))
                outsb = finp.tile([64, 128], f32, tag="outsb", name="outsb")
                nc.scalar.activation(outsb[:], psP[:], AF.Identity,
                                     bias=bpk[0:64, 16:17])
                nc.sync.dma_start(out=outT[:, rb * 128:(rb + 1) * 128],
                                  in_=outsb[:])

            # software pipeline: head and tail are generators; tail steps of
            # older units are interleaved between the MLP chunks of newer
            # units so aggregation work never bunches up on any engine
            units = [(rb, *grp) for rb in range(NRB) for grp in GRP]
            import collections as _c
            tails = _c.deque()

            def pump_tail():
                while tails:
                    try:
                        next(tails[0])
                        return
                    except StopIteration:
                        tails.popleft()

            for u in units:
                for _ in emit_head(*u):
                    if len(tails) >= TAILLAG:
                        pump_tail()
                tails.append(emit_tail(*u))
            while tails:
                pump_tail()

        if loop:
            with tc.For_i(0, loop, 1):
                _emit_all()
        else:
            _emit_all()

    nc.compile()
    return nc


def _prepare(inputs):
    import ml_dtypes
    bf16 = ml_dtypes.bfloat16
    hp = _host_prep(inputs["embeddings"], inputs["src"], inputs["dst"],
                    inputs["bond"])
    wts = _weights_prep(inputs)
    Lb, R, TPB = hp["Lb"], hp["R"], hp["TPB"]
    f32 = np.float32

    # b2slot [C,128,NRB,TPB,H] holds bond ids; b2 index = (bond, head);
    # shipped head-major as [128, NRB, H, TPB]
    b2e = wts["b2"][hp["b2slot"], np.arange(H)[None, None, None, None, :]]
    b2e = np.ascontiguousarray(b2e.transpose(0, 1, 2, 4, 3))

    bpk = np.zeros((128, B * 2 * 2 + 1), f32)
    bpk[:, 0:8] = wts["b0all"]
    bpk[:, 8:16] = wts["b1all"]
    bpk[0:64, 16:17] = wts["biascol"]

    WBF, WBFW = _wbf_layout(TPB)
    wbf = np.zeros((128, WBFW), bf16)
    o, w = WBF["w2all"]
    wbf[:, o:o + w] = wts["w2all"].astype(bf16)
    o, w = WBF["pw4"]
    wbf[0:64, o:o + w] = wts["pw4"].astype(bf16)
    o, w = WBF["id128"]
    wbf[:, o:o + w] = wts["id128"].astype(bf16)

    w0b0 = np.ascontiguousarray(wts["w0all"][:, 0:256]).astype(bf16)
    w0r = np.ascontiguousarray(wts["w0all"][:, 256:1024]).astype(bf16)
    w1b0 = np.ascontiguousarray(wts["w1all"][:, 0:256]).astype(bf16)
    w1r = np.ascontiguousarray(wts["w1all"][:, 256:1024]).astype(bf16)

    key = (Lb, R)
    if key not in _cache:
        _cache.clear()
        _cache[key] = _build_program(Lb, R)
    nc = _cache[key]
    # pack [xe | mask | vge] per rowblock into one contiguous stream
    SEG = R + TPB * 128 + 64 * TPB
    in_maps = []
    for c in range(C):
        strm = np.empty((128, NRB, SEG), bf16)
        xe = hp["xembT"][c].reshape(128, NRB, R)
        mk = hp["maskh"][c].reshape(128, NRB, TPB * 128)
        vg = hp["vge"][c].reshape(128, NRB, 64 * TPB)
        strm[:, :, 0:R] = xe
        strm[:, :, R:R + TPB * 128] = mk
        strm[:, :, R + TPB * 128:SEG] = vg
        m = {"strm": strm.reshape(128, -1),
             "b2e": b2e[c].reshape(128, -1).astype(f32),
             "w0b0": w0b0, "w0r": w0r, "w1b0": w1b0, "w1r": w1r,
             "bpk": bpk, "wbf": wbf}
        in_maps.append(m)
    return nc, in_maps, hp


def kernel(**inputs):
    from concourse.bass_utils import run_bass_kernel_spmd

    nc, in_maps, hp = _prepare(inputs)
    res = run_bass_kernel_spmd(nc, in_maps, list(range(C)))
    out = np.empty((N, D), np.float32)
    rowmap = hp["rowmap"].reshape(C, RPC)
    for c in range(C):
        out[rowmap[c]] = res.results[c]["outT"].T
    return out


def benchmark(inputs, iters=10, warmup=2):
    """Time repeated executions of the compiled SPMD program with
    device-resident inputs (excludes compile and host<->device transfer)."""
    import time
    import jax
    from jax.experimental.shard_map import shard_map
    from jax.sharding import Mesh, PartitionSpec, NamedSharding
    from concourse import bass2jax as b2j
    from concourse import mybir

    nc, in_maps, _hp = _prepare(inputs)
    b2j.install_neuronx_cc_hook()
    partition_name = (nc.partition_id_tensor.name
                      if nc.partition_id_tensor else None)
    in_names, out_names, out_avals, zero_outs = [], [], [], []
    for alloc in nc.m.functions[0].allocations:
        if not isinstance(alloc, mybir.MemoryLocationSet):
            continue
        name = alloc.memorylocations[0].name
        if alloc.kind == "ExternalInput":
            if name != partition_name:
                in_names.append(name)
        elif alloc.kind == "ExternalOutput":
            out_names.append(name)
            shape = tuple(alloc.tensor_shape)
            dtype = mybir.dt.np(alloc.dtype)
            out_avals.append(jax.core.ShapedArray(shape, dtype))
            zero_outs.append(np.zeros(shape, dtype))
    n_params = len(in_names)
    all_in = in_names + out_names + ([partition_name] if partition_name
                                     else [])
    donate = tuple(range(n_params, n_params + len(out_names)))

    def _body(*args):
        operands = list(args)
        if partition_name is not None:
            operands.append(b2j.partition_id_tensor())
        outs = b2j._bass_exec_p.bind(
            *operands, out_avals=tuple(out_avals), in_names=tuple(all_in),
            out_names=tuple(out_names), lowering_input_output_aliases=(),
            sim_require_finite=True, sim_require_nnan=True, nc=nc)
        return tuple(outs)

    devices = jax.devices()[:C]
    mesh = Mesh(np.asarray(devices), ("core",))
    in_specs = (PartitionSpec("core"),) * (n_params + len(out_names))
    out_specs = (PartitionSpec("core"),) * len(out_names)
    sharded = jax.jit(shard_map(_body, mesh=mesh, in_specs=in_specs,
                                out_specs=out_specs, check_rep=False),
                      donate_argnums=donate, keep_unused=True)
    sh = NamedSharding(mesh, PartitionSpec("core"))
    concat_in = [
        jax.device_put(
            np.concatenate([np.asarray(in_maps[c][n]) for c in range(C)],
                           axis=0), sh)
        for n in in_names]

    times = []
    for it in range(warmup + iters):
        zs = [jax.device_put(np.zeros((C * z.shape[0], *z.shape[1:]),
                                      z.dtype), sh)
              for z in zero_outs]
        t0 = time.perf_counter()
        out = sharded(*concat_in, *zs)
        jax.block_until_ready(out)
        dt = time.perf_counter() - t0
        if it >= warmup:
            times.append(dt)
    print("bench times (ms):", [f"{t*1e3:.3f}" for t in times])
    return min(times) * 1e9


def benchmark_hw(inputs, k=512, iters=6, warmup=2, k_small=None):
    """Real-HW timing: run the whole per-core program k times inside one
    NEFF (tc.For_i) and wall-time it through the tunnel. If k_small is
    given, also times a k_small-loop NEFF and returns the difference
    quotient, which cancels the (~80ms) tunnel dispatch floor exactly."""
    if k_small:
        t_big = benchmark_hw(inputs, k=k, iters=iters, warmup=warmup)
        t_sml = benchmark_hw(inputs, k=k_small, iters=iters, warmup=warmup)
        return (t_big * k - t_sml * k_small) / (k - k_small)
    import time
    import jax
    from jax.experimental.shard_map import shard_map
    from jax.sharding import Mesh, PartitionSpec, NamedSharding
    from concourse import bass2jax as b2j
    from concourse import mybir

    nc0, in_maps, hp = _prepare(inputs)
    nc = _build_program(hp["Lb"], hp["R"], loop=k)

    b2j.install_neuronx_cc_hook()
    partition_name = (nc.partition_id_tensor.name
                      if nc.partition_id_tensor else None)
    in_names, out_names, out_avals, zero_outs = [], [], [], []
    for alloc in nc.m.functions[0].allocations:
        if not isinstance(alloc, mybir.MemoryLocationSet):
            continue
        name = alloc.memorylocations[0].name
        if alloc.kind == "ExternalInput":
            if name != partition_name:
                in_names.append(name)
        elif alloc.kind == "ExternalOutput":
            out_names.append(name)
            shape = tuple(alloc.tensor_shape)
            dtype = mybir.dt.np(alloc.dtype)
            out_avals.append(jax.core.ShapedArray(shape, dtype))
            zero_outs.append(np.zeros(shape, dtype))
    n_params = len(in_names)
    all_in = in_names + out_names + ([partition_name] if partition_name
                                     else [])
    donate = tuple(range(n_params, n_params + len(out_names)))

    def _body(*args):
        operands = list(args)
        if partition_name is not None:
            operands.append(b2j.partition_id_tensor())
        outs = b2j._bass_exec_p.bind(
            *operands, out_avals=tuple(out_avals), in_names=tuple(all_in),
            out_names=tuple(out_names), lowering_input_output_aliases=(),
            sim_require_finite=True, sim_require_nnan=True, nc=nc)
        return tuple(outs)

    devices = jax.devices()[:C]
    mesh = Mesh(np.asarray(devices), ("core",))
    in_specs = (PartitionSpec("core"),) * (n_params + len(out_names))
    out_specs = (PartitionSpec("core"),) * len(out_names)
    sharded = jax.jit(shard_map(_body, mesh=mesh, in_specs=in_specs,
                                out_specs=out_specs, check_rep=False),
                      donate_argnums=donate, keep_unused=True)
    sh = NamedSharding(mesh, PartitionSpec("core"))
    concat_in = [
        jax.device_put(
            np.concatenate([np.asarray(in_maps[c][n]) for c in range(C)],
                           axis=0), sh)
        for n in in_names]
    times = []
    for it in range(warmup + iters):
        zs = [jax.device_put(np.zeros((C * z.shape[0], *z.shape[1:]),
                                      z.dtype), sh)
              for z in zero_outs]
        t0 = time.perf_counter()
        out = sharded(*concat_in, *zs)
        jax.block_until_ready(out)
        dt = time.perf_counter() - t0
        if it >= warmup:
            times.append(dt)
    print("looped bench times (ms):", [f"{t*1e3:.2f}" for t in times])
    best = min(times)
    return best * 1e9 / k
